# revision 24
# baseline (speedup 1.0000x reference)
"""Bass/Trainium2 kernel for nn_LocalAggregator (GNN message passing).

Math per batch b (hidden [64,128], adj [64,64] in {0..4}, a [4,128]):
    e_k[i,j] = leakyrelu_{0.2}( sum_d hidden[i,d]*hidden[j,d]*a[k,d] )
    alpha    = softmax_j( where(adj==k+1, e_k, -9e15) )
    out      = alpha @ hidden

Device strategy (8 cores, pure batch data-parallel, 64 batches/core,
processed in "octs" of 8 batches):
  - e_k is SYMMETRIC in (i,j): the PSUM tile holding e_k[i,j] doubles as
    e_k[j,i], so masking with the host-TRANSPOSED adjacency yields the
    transposed attention weights directly -- no on-chip transposes.
  - The one-hot selection masks ind_k = (adjT == k+1) are precomputed on
    the HOST and shipped as bf16 (DMA has bandwidth headroom; the vector
    engine does not).
  - w_all[d,(l,k,j)] = h[j,d]*a[k,d] is ONE DVE tensor_tensor with
    broadcast reads (hT repeated over k, aPat repeated over l).
  - leaky-relu runs on ACT as Prelu(0.2) evacuating PSUM; Exp follows.
    GpSimd is NOT used for element-wise work: it shares an SBUF port
    with the vector engine (concurrent ops slow DVE ~2.4x, measured).
  - A ones-column appended to hidden makes the output matmul emit the
    softmax denominator.  The out PSUM tile puts each pair-block at a
    256-col offset (bank-aligned); evacuation is split between ACT and
    DVE to balance them; the division happens on the HOST.
  - Dummy matmuls at startup trip the HAM clock gate (1.2 -> 2.4 GHz)
    during the otherwise-idle preamble.
"""

import numpy as np
import ml_dtypes

from contextlib import ExitStack

import concourse.bass as bass
import concourse.tile as tile
from concourse import bacc, mybir
from concourse._compat import with_exitstack
from concourse.bass_utils import run_bass_kernel_spmd

BF16 = mybir.dt.bfloat16
F32 = mybir.dt.float32
ALU = mybir.AluOpType
ACTF = mybir.ActivationFunctionType

B, N, D, K = 512, 64, 128, 4
NCORES = 8
BPC = B // NCORES          # 64 batches per core
OCTS = BPC // 8            # 8 octs of 8 batches per core
HHW = 132                  # hidden cols + ones col + pad (128 data, 1 ones, 3 zero)
OW = 520                   # output cols: 512 bf16 num + 4 f32 denom (as 8 bf16)
INW = 512 + 1024 + 4 * HHW  # packed input cols: hT | ind4 | hh


@with_exitstack
def _kernel_body(ctx, tc, in_d, aPat_d, out_d):
    nc = tc.nc

    const_pool = ctx.enter_context(tc.tile_pool(name="const", bufs=1))
    in_pool = ctx.enter_context(tc.tile_pool(name="inp", bufs=6))
    work_pool = ctx.enter_context(tc.tile_pool(name="work", bufs=5))
    psum_pool = ctx.enter_context(tc.tile_pool(name="psum", bufs=3, space="PSUM"))
    opsum_pool = ctx.enter_context(tc.tile_pool(name="opsum", bufs=1, space="PSUM"))

    # --- one-time constants ---
    # aPat[d, k*64+j] = a[k,d]  (host-precomputed)
    aPat = const_pool.tile([128, 256], BF16)
    nc.sync.dma_start(out=aPat[:], in_=aPat_d[:, :])

    # PE warm-up: dummy matmuls during the (otherwise idle) startup
    # window trip the HAM clock gate (1.2 -> 2.4 GHz) so the real
    # matmuls run warm from the first oct.
    warm = const_pool.tile([128, 512], BF16)
    nc.gpsimd.memset(warm[:], 0.0)
    wps = psum_pool.tile([128, 1024], F32, tag="e4")
    for _ in range(8):
        nc.tensor.matmul(wps[0:8, 0:512], lhsT=warm[:, 0:8], rhs=warm[:],
                         start=True, stop=True)

    for q in range(OCTS):
        # ---- packed load, split so the critical hT part lands first ----
        inp = in_pool.tile([128, INW], BF16, tag="inp")
        nc.sync.dma_start(out=inp[:], in_=in_d[q])
        hT = inp[:, 0:512]
        ind = inp[:, 512:1536]
        hh = inp[:, 1536:INW]

        # ---- w_all[d, (l,k,j)] = hT[d, (l,j)] * a[k,d] : one DVE op ----
        w_all = work_pool.tile([128, 2048], BF16, tag="w_all")
        w_allv = w_all[:].rearrange("p (l k j) -> p l k j", l=8, k=4)
        hTv = (hT.rearrange("p (l j) -> p l j", l=8)
               .unsqueeze(2).broadcast_to([128, 8, 4, 64]))
        aPatv = (aPat[:].rearrange("p (k j) -> p k j", k=4)
                 .unsqueeze(1).broadcast_to([128, 8, 4, 64]))
        nc.vector.tensor_tensor(w_allv, hTv, aPatv, ALU.mult)

        # ---- e4[(u,i), (c,k,j)] = e_k^{l=2c+u}[i,j] : 8 matmuls ----
        e4 = psum_pool.tile([128, 1024], F32, tag="e4")
        for l in range(8):
            c, u = l // 2, l % 2
            nc.tensor.matmul(
                e4[u * 64 : (u + 1) * 64, c * 256 : (c + 1) * 256],
                lhsT=hT[:, l * 64 : (l + 1) * 64],
                rhs=w_all[:, l * 256 : (l + 1) * 256],
                start=True, stop=True,
                tile_position=(0, u * 64),
            )

        # ---- xm = exp(leakyrelu(e)) : Prelu evacuates PSUM, then Exp ----
        lr4 = work_pool.tile([128, 1024], F32, tag="lr4")
        nc.scalar.activation(lr4[:], e4[:], ACTF.Prelu, alpha=0.2)
        xm = work_pool.tile([128, 1024], BF16, tag="xm")
        nc.scalar.activation(xm[:], lr4[:], ACTF.Exp)

        # ---- one-hot select (host-precomputed masks, symmetry trick) ----
        w4 = work_pool.tile([128, 1024], BF16, tag="w4")
        nc.vector.tensor_mul(w4[:], xm[:], ind)

        # ---- partial sum over k: t2[(u,j), (c,k2,i)] ----
        w4v = w4[:].rearrange("p (c k s) -> p c k s", c=4, k=4)
        t2 = work_pool.tile([128, 512], BF16, tag="t2")
        t2v = t2[:].rearrange("p (c k s) -> p c k s", c=4, k=2)
        nc.vector.tensor_tensor(t2v, w4v[:, :, 0:2, :], w4v[:, :, 2:4, :], ALU.add)

        # ---- out[(u,i), (c,:)] = sum_j w^T[j,i] hh[j,:]; col 128 = denom ----
        # the remaining k-pair sum rides on PSUM accumulation (2 matmuls).
        # pair-blocks live at 256-col offsets so no matmul output crosses
        # a 2 KiB PSUM bank.
        ops = opsum_pool.tile([128, 1024], F32, tag="ops")
        for l in range(8):
            c, u = l // 2, l % 2
            for h in range(2):
                nc.tensor.matmul(
                    ops[u * 64 : (u + 1) * 64, c * 256 : c * 256 + HHW],
                    lhsT=t2[u * 64 : (u + 1) * 64,
                            c * 128 + h * 64 : c * 128 + (h + 1) * 64],
                    rhs=hh[u * 64 : (u + 1) * 64, c * HHW : (c + 1) * HHW],
                    start=(h == 0), stop=(h == 1),
                    tile_position=(u * 64, u * 64),
                )

        # ---- numerator (bf16) + denominator (f32 bits smuggled into the
        #      same bf16 tile via bitcast) to HBM; host divides.
        #      evacuation split between ACT and DVE to balance load. ----
        if q % 2 == 0:
            osb2 = work_pool.tile([128, 2 * OW], BF16, tag="osb2")
        osb = osb2[:, (q % 2) * OW : (q % 2 + 1) * OW]
        nc.scalar.activation(
            osb[:, 0:256].rearrange("p (c w) -> p c w", c=2),
            ops[:, 0:512].rearrange("p (c z) -> p c z", c=2)[:, :, 0:128],
            ACTF.Copy)
        nc.vector.tensor_copy(
            osb[:, 256:512].rearrange("p (c w) -> p c w", c=2),
            ops[:, 512:1024].rearrange("p (c z) -> p c z", c=2)[:, :, 0:128])
        nc.scalar.activation(
            osb[:, 512:520].bitcast(F32),
            ops[:].rearrange("p (c z) -> p c z", c=4)[:, :, 128],
            ACTF.Copy)
        if q % 2 == 1:
            nc.sync.dma_start(
                out=out_d[q - 1 : q + 1].rearrange("g p w -> p g w"),
                in_=osb2[:].rearrange("p (g w) -> p g w", g=2))


def build_nc():
    nc = bacc.Bacc("TRN2", target_bir_lowering=False, debug=False)
    in_d = nc.dram_tensor("inp", [OCTS, 128, INW], BF16, kind="ExternalInput").ap()
    aPat_d = nc.dram_tensor("apat", [128, 256], BF16, kind="ExternalInput").ap()
    out_d = nc.dram_tensor("out", [OCTS, 128, OW], BF16, kind="ExternalOutput").ap()
    with tile.TileContext(nc) as tc:
        _kernel_body(tc, in_d, aPat_d, out_d)
    nc.compile()
    return nc


def prep_inputs(hidden, adj, a):
    """Host-side packing: bf16 casts, transposed/interleaved layouts, shards."""
    bf = ml_dtypes.bfloat16
    hidden = np.asarray(hidden, dtype=np.float32)
    adj = np.asarray(adj)
    a = np.asarray(a, dtype=np.float32)

    hb = hidden.astype(bf)                                   # [B, 64, 128]

    # hT_q[q, d, l*64+i] = hidden[8q+l, i, d]
    hT = (hb.transpose(0, 2, 1)                              # [B, d, i]
          .reshape(B // 8, 8, D, N)                          # [q, l, d, i]
          .transpose(0, 2, 1, 3)                             # [q, d, l, i]
          .reshape(B // 8, D, 8 * N))

    # ind4_q[q, u*64+r, c*256+k*64+s] = (adj[8q+2c+u][s, r] == k+1)
    adjT = adj.transpose(0, 2, 1)                            # [b, r, s]
    onehot = (adjT[:, :, None, :] == np.arange(1, K + 1)[None, None, :, None]
              ).astype(bf)                                   # [b, r, k, s]
    ind4 = (onehot.reshape(B // 8, 4, 2, N, K, N)            # [q, c, u, r, k, s]
            .transpose(0, 2, 3, 1, 4, 5)                     # [q, u, r, c, k, s]
            .reshape(B // 8, 2 * N, 4 * K * N))

    # hh_oct[q][u*64+j, c*132+d] = hidden[8q+2c+u, j, d]; col 128 = 1
    hh = np.zeros((B, N, HHW), dtype=bf)
    hh[:, :, 0:D] = hb
    hh[:, :, D] = bf(1.0)
    hhq = (hh.reshape(B // 8, 4, 2, N, HHW)                  # [q, c, u, j, :]
           .transpose(0, 2, 3, 1, 4)                         # [q, u, j, c, :]
           .reshape(B // 8, 2 * N, 4 * HHW))

    packed = np.concatenate([hT, ind4, hhq], axis=2)         # [B//8, 128, INW]
    packed = np.ascontiguousarray(packed)

    # aPat[d, k*64+j] = a[k, d]
    aPat = np.ascontiguousarray(
        np.broadcast_to(a.T[:, :, None], (D, K, N)).reshape(D, K * N)
    ).astype(bf)

    in_maps = []
    for cidx in range(NCORES):
        qsl = slice(cidx * OCTS, (cidx + 1) * OCTS)
        in_maps.append({
            "inp": np.ascontiguousarray(packed[qsl]),
            "apat": aPat,
        })
    return in_maps


_NC_CACHE = {}


def run_device(hidden, adj, a, **spmd_kwargs):
    if "nc" not in _NC_CACHE:
        _NC_CACHE["nc"] = build_nc()
    nc = _NC_CACHE["nc"]
    in_maps = prep_inputs(hidden, adj, a)
    res = run_bass_kernel_spmd(nc, in_maps, list(range(NCORES)), **spmd_kwargs)
    raw = np.stack([res.results[c]["out"] for c in range(NCORES)], axis=0)
    raw = raw.reshape(NCORES * OCTS, 128, OW)                # [q, (u,i), :]
    num = raw[:, :, 0:512].astype(np.float32)                # [q, (u,i), (c,d)]
    den = np.ascontiguousarray(raw[:, :, 512:520]).view(np.float32)  # [q,(u,i),c]
    o = num.reshape(NCORES * OCTS, 2, N, 4, D).transpose(0, 3, 1, 2, 4)
    s = den.reshape(NCORES * OCTS, 2, N, 4).transpose(0, 3, 1, 2)
    out = (o / s[..., None]).reshape(B, N, D)
    return np.ascontiguousarray(out, dtype=np.float32), res


def kernel(hidden, adj, a):
    out, _ = run_device(hidden, adj, a)
    return out


# revision 25
# speedup vs baseline: 1.1021x; 1.1021x over previous
"""Bass/Trainium2 kernel for nn_LocalAggregator (GNN message passing).

Math per batch b (hidden [64,128], adj [64,64] in {0..4}, a [4,128]):
    e_k[i,j] = leakyrelu_{0.2}( sum_d hidden[i,d]*hidden[j,d]*a[k,d] )
    alpha    = softmax_j( where(adj==k+1, e_k, -9e15) )
    out      = alpha @ hidden

Device strategy (8 cores, pure batch data-parallel, 64 batches/core,
processed in "octs" of 8 batches):
  - e_k is SYMMETRIC in (i,j): the PSUM tile holding e_k[i,j] doubles as
    e_k[j,i], so masking with the host-TRANSPOSED adjacency yields the
    transposed attention weights directly -- no on-chip transposes.
  - The one-hot selection masks ind_k = (adjT == k+1) are precomputed on
    the HOST and shipped as bf16 (DMA has bandwidth headroom; the vector
    engine does not).
  - w_all[d,(l,k,j)] = h[j,d]*a[k,d] is ONE DVE tensor_tensor with
    broadcast reads (hT repeated over k, aPat repeated over l).
  - leaky-relu runs on ACT as Prelu(0.2) evacuating PSUM; Exp follows.
    GpSimd is NOT used for element-wise work: it shares an SBUF port
    with the vector engine (concurrent ops slow DVE ~2.4x, measured).
  - A ones-column appended to hidden makes the output matmul emit the
    softmax denominator.  The out PSUM tile puts each pair-block at a
    256-col offset (bank-aligned); evacuation is split between ACT and
    DVE to balance them; the division happens on the HOST.
  - Dummy matmuls at startup trip the HAM clock gate (1.2 -> 2.4 GHz)
    during the otherwise-idle preamble.
"""

import numpy as np
import ml_dtypes

from contextlib import ExitStack

import concourse.bass as bass
import concourse.tile as tile
from concourse import bacc, mybir
from concourse._compat import with_exitstack
from concourse.bass_utils import run_bass_kernel_spmd

BF16 = mybir.dt.bfloat16
F32 = mybir.dt.float32
ALU = mybir.AluOpType
ACTF = mybir.ActivationFunctionType

B, N, D, K = 512, 64, 128, 4
NCORES = 8
BPC = B // NCORES          # 64 batches per core
OCTS = BPC // 8            # 8 octs of 8 batches per core
HHW = 132                  # hidden cols + ones col + pad (128 data, 1 ones, 3 zero)
OW = 520                   # output cols: 512 bf16 num + 4 f32 denom (as 8 bf16)
INW = 512 + 1024 + 4 * HHW  # packed input cols: hT | ind4 | hh


@with_exitstack
def _kernel_body(ctx, tc, in_d, aPat_d, out_d):
    nc = tc.nc

    const_pool = ctx.enter_context(tc.tile_pool(name="const", bufs=1))
    in_pool = ctx.enter_context(tc.tile_pool(name="inp", bufs=6))
    work_pool = ctx.enter_context(tc.tile_pool(name="work", bufs=5))
    psum_pool = ctx.enter_context(tc.tile_pool(name="psum", bufs=2, space="PSUM"))
    opsum_pool = ctx.enter_context(tc.tile_pool(name="opsum", bufs=2, space="PSUM"))

    # --- one-time constants ---
    # aPat[d, k*64+j] = a[k,d]  (host-precomputed)
    aPat = const_pool.tile([128, 256], BF16)
    nc.sync.dma_start(out=aPat[:], in_=aPat_d[:, :])

    # PE warm-up: dummy matmuls during the (otherwise idle) startup
    # window trip the HAM clock gate (1.2 -> 2.4 GHz) so the real
    # matmuls run warm from the first oct.
    warm = const_pool.tile([128, 512], BF16)
    nc.gpsimd.memset(warm[:], 0.0)
    wps = psum_pool.tile([128, 1024], F32, tag="e4")
    for _ in range(8):
        nc.tensor.matmul(wps[0:8, 0:512], lhsT=warm[:, 0:8], rhs=warm[:],
                         start=True, stop=True)

    for q in range(OCTS):
        # ---- packed load, split so the critical hT part lands first ----
        inp = in_pool.tile([128, INW], BF16, tag="inp")
        nc.sync.dma_start(out=inp[:, 0:512], in_=in_d[q][:, 0:512])
        nc.sync.dma_start(out=inp[:, 512:INW], in_=in_d[q][:, 512:INW])
        hT = inp[:, 0:512]
        ind = inp[:, 512:1536]
        hh = inp[:, 1536:INW]

        # ---- w_all[d, (l,k,j)] = hT[d, (l,j)] * a[k,d] : one DVE op ----
        w_all = work_pool.tile([128, 2048], BF16, tag="w_all")
        w_allv = w_all[:].rearrange("p (l k j) -> p l k j", l=8, k=4)
        hTv = (hT.rearrange("p (l j) -> p l j", l=8)
               .unsqueeze(2).broadcast_to([128, 8, 4, 64]))
        aPatv = (aPat[:].rearrange("p (k j) -> p k j", k=4)
                 .unsqueeze(1).broadcast_to([128, 8, 4, 64]))
        nc.vector.tensor_tensor(w_allv, hTv, aPatv, ALU.mult)

        # ---- e4[(u,i), (c,k,j)] = e_k^{l=2c+u}[i,j] : 8 matmuls ----
        e4 = psum_pool.tile([128, 1024], F32, tag="e4")
        for l in range(8):
            c, u = l // 2, l % 2
            nc.tensor.matmul(
                e4[u * 64 : (u + 1) * 64, c * 256 : (c + 1) * 256],
                lhsT=hT[:, l * 64 : (l + 1) * 64],
                rhs=w_all[:, l * 256 : (l + 1) * 256],
                start=True, stop=True,
                tile_position=(0, u * 64),
            )

        # ---- xm = exp(leakyrelu(e)) : Prelu evacuates PSUM, then Exp ----
        lr4 = work_pool.tile([128, 1024], F32, tag="lr4")
        nc.scalar.activation(lr4[:], e4[:], ACTF.Prelu, alpha=0.2)
        xm = work_pool.tile([128, 1024], BF16, tag="xm")
        nc.scalar.activation(xm[:], lr4[:], ACTF.Exp)

        # ---- one-hot select (host-precomputed masks, symmetry trick) ----
        w4 = work_pool.tile([128, 1024], BF16, tag="w4")
        nc.vector.tensor_mul(w4[:], xm[:], ind)

        # ---- partial sum over k: t2[(u,j), (c,k2,i)] ----
        w4v = w4[:].rearrange("p (c k s) -> p c k s", c=4, k=4)
        t2 = work_pool.tile([128, 512], BF16, tag="t2")
        t2v = t2[:].rearrange("p (c k s) -> p c k s", c=4, k=2)
        nc.vector.tensor_tensor(t2v, w4v[:, :, 0:2, :], w4v[:, :, 2:4, :], ALU.add)

        # ---- out[(u,i), (c,:)] = sum_j w^T[j,i] hh[j,:]; col 128 = denom ----
        # the remaining k-pair sum rides on PSUM accumulation (2 matmuls).
        # pair-blocks live at 256-col offsets so no matmul output crosses
        # a 2 KiB PSUM bank.
        ops = opsum_pool.tile([128, 1024], F32, tag="ops")
        for l in range(8):
            c, u = l // 2, l % 2
            for h in range(2):
                nc.tensor.matmul(
                    ops[u * 64 : (u + 1) * 64, c * 256 : c * 256 + HHW],
                    lhsT=t2[u * 64 : (u + 1) * 64,
                            c * 128 + h * 64 : c * 128 + (h + 1) * 64],
                    rhs=hh[u * 64 : (u + 1) * 64, c * HHW : (c + 1) * HHW],
                    start=(h == 0), stop=(h == 1),
                    tile_position=(u * 64, u * 64),
                )

        # ---- numerator (bf16) + denominator (f32 bits smuggled into the
        #      same bf16 tile via bitcast) to HBM; host divides.
        #      evacuation split between ACT and DVE to balance load. ----
        if q % 2 == 0:
            osb2 = work_pool.tile([128, 2 * OW], BF16, tag="osb2")
        osb = osb2[:, (q % 2) * OW : (q % 2 + 1) * OW]
        nc.scalar.activation(
            osb[:, 0:256].rearrange("p (c w) -> p c w", c=2),
            ops[:, 0:512].rearrange("p (c z) -> p c z", c=2)[:, :, 0:128],
            ACTF.Copy)
        nc.vector.tensor_copy(
            osb[:, 256:512].rearrange("p (c w) -> p c w", c=2),
            ops[:, 512:1024].rearrange("p (c z) -> p c z", c=2)[:, :, 0:128])
        nc.scalar.activation(
            osb[:, 512:520].bitcast(F32),
            ops[:].rearrange("p (c z) -> p c z", c=4)[:, :, 128],
            ACTF.Copy)
        if q % 2 == 1:
            nc.sync.dma_start(
                out=out_d[q - 1 : q + 1].rearrange("g p w -> p g w"),
                in_=osb2[:].rearrange("p (g w) -> p g w", g=2))


def build_nc():
    nc = bacc.Bacc("TRN2", target_bir_lowering=False, debug=False)
    in_d = nc.dram_tensor("inp", [OCTS, 128, INW], BF16, kind="ExternalInput").ap()
    aPat_d = nc.dram_tensor("apat", [128, 256], BF16, kind="ExternalInput").ap()
    out_d = nc.dram_tensor("out", [OCTS, 128, OW], BF16, kind="ExternalOutput").ap()
    with tile.TileContext(nc) as tc:
        _kernel_body(tc, in_d, aPat_d, out_d)
    nc.compile()
    return nc


def prep_inputs(hidden, adj, a):
    """Host-side packing: bf16 casts, transposed/interleaved layouts, shards."""
    bf = ml_dtypes.bfloat16
    hidden = np.asarray(hidden, dtype=np.float32)
    adj = np.asarray(adj)
    a = np.asarray(a, dtype=np.float32)

    hb = hidden.astype(bf)                                   # [B, 64, 128]

    # hT_q[q, d, l*64+i] = hidden[8q+l, i, d]
    hT = (hb.transpose(0, 2, 1)                              # [B, d, i]
          .reshape(B // 8, 8, D, N)                          # [q, l, d, i]
          .transpose(0, 2, 1, 3)                             # [q, d, l, i]
          .reshape(B // 8, D, 8 * N))

    # ind4_q[q, u*64+r, c*256+k*64+s] = (adj[8q+2c+u][s, r] == k+1)
    adjT = adj.transpose(0, 2, 1)                            # [b, r, s]
    onehot = (adjT[:, :, None, :] == np.arange(1, K + 1)[None, None, :, None]
              ).astype(bf)                                   # [b, r, k, s]
    ind4 = (onehot.reshape(B // 8, 4, 2, N, K, N)            # [q, c, u, r, k, s]
            .transpose(0, 2, 3, 1, 4, 5)                     # [q, u, r, c, k, s]
            .reshape(B // 8, 2 * N, 4 * K * N))

    # hh_oct[q][u*64+j, c*132+d] = hidden[8q+2c+u, j, d]; col 128 = 1
    hh = np.zeros((B, N, HHW), dtype=bf)
    hh[:, :, 0:D] = hb
    hh[:, :, D] = bf(1.0)
    hhq = (hh.reshape(B // 8, 4, 2, N, HHW)                  # [q, c, u, j, :]
           .transpose(0, 2, 3, 1, 4)                         # [q, u, j, c, :]
           .reshape(B // 8, 2 * N, 4 * HHW))

    packed = np.concatenate([hT, ind4, hhq], axis=2)         # [B//8, 128, INW]
    packed = np.ascontiguousarray(packed)

    # aPat[d, k*64+j] = a[k, d]
    aPat = np.ascontiguousarray(
        np.broadcast_to(a.T[:, :, None], (D, K, N)).reshape(D, K * N)
    ).astype(bf)

    in_maps = []
    for cidx in range(NCORES):
        qsl = slice(cidx * OCTS, (cidx + 1) * OCTS)
        in_maps.append({
            "inp": np.ascontiguousarray(packed[qsl]),
            "apat": aPat,
        })
    return in_maps


_NC_CACHE = {}


def run_device(hidden, adj, a, **spmd_kwargs):
    if "nc" not in _NC_CACHE:
        _NC_CACHE["nc"] = build_nc()
    nc = _NC_CACHE["nc"]
    in_maps = prep_inputs(hidden, adj, a)
    res = run_bass_kernel_spmd(nc, in_maps, list(range(NCORES)), **spmd_kwargs)
    raw = np.stack([res.results[c]["out"] for c in range(NCORES)], axis=0)
    raw = raw.reshape(NCORES * OCTS, 128, OW)                # [q, (u,i), :]
    num = raw[:, :, 0:512].astype(np.float32)                # [q, (u,i), (c,d)]
    den = np.ascontiguousarray(raw[:, :, 512:520]).view(np.float32)  # [q,(u,i),c]
    o = num.reshape(NCORES * OCTS, 2, N, 4, D).transpose(0, 3, 1, 2, 4)
    s = den.reshape(NCORES * OCTS, 2, N, 4).transpose(0, 3, 1, 2)
    out = (o / s[..., None]).reshape(B, N, D)
    return np.ascontiguousarray(out, dtype=np.float32), res


def kernel(hidden, adj, a):
    out, _ = run_device(hidden, adj, a)
    return out


# revision 26
# speedup vs baseline: 1.1452x; 1.0391x over previous
"""Bass/Trainium2 kernel for nn_LocalAggregator (GNN message passing).

Math per batch b (hidden [64,128], adj [64,64] in {0..4}, a [4,128]):
    e_k[i,j] = leakyrelu_{0.2}( sum_d hidden[i,d]*hidden[j,d]*a[k,d] )
    alpha    = softmax_j( where(adj==k+1, e_k, -9e15) )
    out      = alpha @ hidden

Device strategy (8 cores, pure batch data-parallel, 64 batches/core,
processed in "octs" of 8 batches):
  - e_k is SYMMETRIC in (i,j): the PSUM tile holding e_k[i,j] doubles as
    e_k[j,i], so masking with the host-TRANSPOSED adjacency yields the
    transposed attention weights directly -- no on-chip transposes.
  - The one-hot selection masks ind_k = (adjT == k+1) are precomputed on
    the HOST and shipped as bf16 (DMA has bandwidth headroom; the vector
    engine does not).
  - w_all[d,(l,k,j)] = h[j,d]*a[k,d] is ONE DVE tensor_tensor with
    broadcast reads (hT repeated over k, aPat repeated over l).
  - leaky-relu runs on ACT as Prelu(0.2) evacuating PSUM; Exp follows.
    GpSimd is NOT used for element-wise work: it shares an SBUF port
    with the vector engine (concurrent ops slow DVE ~2.4x, measured).
  - A ones-column appended to hidden makes the output matmul emit the
    softmax denominator.  The out PSUM tile puts each pair-block at a
    256-col offset (bank-aligned); evacuation is split between ACT and
    DVE to balance them; the division happens on the HOST.
  - Dummy matmuls at startup trip the HAM clock gate (1.2 -> 2.4 GHz)
    during the otherwise-idle preamble.
"""

import numpy as np
import ml_dtypes

from contextlib import ExitStack

import concourse.bass as bass
import concourse.tile as tile
from concourse import bacc, mybir
from concourse._compat import with_exitstack
from concourse.bass_utils import run_bass_kernel_spmd

BF16 = mybir.dt.bfloat16
F32 = mybir.dt.float32
ALU = mybir.AluOpType
ACTF = mybir.ActivationFunctionType

B, N, D, K = 512, 64, 128, 4
NCORES = 8
BPC = B // NCORES          # 64 batches per core
OCTS = BPC // 8            # 8 octs of 8 batches per core
HHW = 132                  # hidden cols + ones col + pad (128 data, 1 ones, 3 zero)
OW = 520                   # output cols: 512 bf16 num + 4 f32 denom (as 8 bf16)
INW = 512 + 1024 + 4 * HHW  # packed input cols: hT | ind4 | hh


@with_exitstack
def _kernel_body(ctx, tc, in_d, aPat_d, out_d):
    nc = tc.nc

    const_pool = ctx.enter_context(tc.tile_pool(name="const", bufs=1))
    in_pool = ctx.enter_context(tc.tile_pool(name="inp", bufs=6))
    work_pool = ctx.enter_context(tc.tile_pool(name="work", bufs=7))
    psum_pool = ctx.enter_context(tc.tile_pool(name="psum", bufs=2, space="PSUM"))
    opsum_pool = ctx.enter_context(tc.tile_pool(name="opsum", bufs=2, space="PSUM"))

    # --- one-time constants ---
    # aPat[d, k*64+j] = a[k,d]  (host-precomputed)
    aPat = const_pool.tile([128, 256], BF16)
    nc.sync.dma_start(out=aPat[:], in_=aPat_d[:, :])

    # PE warm-up: dummy matmuls during the (otherwise idle) startup
    # window trip the HAM clock gate (1.2 -> 2.4 GHz) so the real
    # matmuls run warm from the first oct.
    warm = const_pool.tile([128, 512], BF16)
    nc.gpsimd.memset(warm[:], 0.0)
    wps = psum_pool.tile([128, 1024], F32, tag="e4")
    for _ in range(5):
        nc.tensor.matmul(wps[0:8, 0:512], lhsT=warm[:, 0:8], rhs=warm[:],
                         start=True, stop=True)

    for q in range(OCTS):
        # ---- packed load, split so the critical hT part lands first ----
        inp = in_pool.tile([128, INW], BF16, tag="inp")
        nc.sync.dma_start(out=inp[:, 0:512], in_=in_d[q][:, 0:512])
        nc.sync.dma_start(out=inp[:, 512:INW], in_=in_d[q][:, 512:INW])
        hT = inp[:, 0:512]
        ind = inp[:, 512:1536]
        hh = inp[:, 1536:INW]

        # ---- w_all[d, (l,k,j)] = hT[d, (l,j)] * a[k,d] : one DVE op ----
        w_all = work_pool.tile([128, 2048], BF16, tag="w_all")
        w_allv = w_all[:].rearrange("p (l k j) -> p l k j", l=8, k=4)
        hTv = (hT.rearrange("p (l j) -> p l j", l=8)
               .unsqueeze(2).broadcast_to([128, 8, 4, 64]))
        aPatv = (aPat[:].rearrange("p (k j) -> p k j", k=4)
                 .unsqueeze(1).broadcast_to([128, 8, 4, 64]))
        nc.vector.tensor_tensor(w_allv, hTv, aPatv, ALU.mult)

        # ---- e4[(u,i), (c,k,j)] = e_k^{l=2c+u}[i,j] : 8 matmuls ----
        e4 = psum_pool.tile([128, 1024], F32, tag="e4")
        for l in range(8):
            c, u = l // 2, l % 2
            nc.tensor.matmul(
                e4[u * 64 : (u + 1) * 64, c * 256 : (c + 1) * 256],
                lhsT=hT[:, l * 64 : (l + 1) * 64],
                rhs=w_all[:, l * 256 : (l + 1) * 256],
                start=True, stop=True,
                tile_position=(0, u * 64),
            )

        # ---- xm = exp(leakyrelu(e)) : Prelu evacuates PSUM, then Exp ----
        lr4 = work_pool.tile([128, 1024], F32, tag="lr4")
        nc.scalar.activation(lr4[:], e4[:], ACTF.Prelu, alpha=0.2)
        xm = work_pool.tile([128, 1024], BF16, tag="xm")
        nc.scalar.activation(xm[:], lr4[:], ACTF.Exp)

        # ---- one-hot select (host-precomputed masks, symmetry trick) ----
        w4 = work_pool.tile([128, 1024], BF16, tag="w4")
        nc.vector.tensor_mul(w4[:], xm[:], ind)

        # ---- partial sum over k: t2[(u,j), (c,k2,i)] ----
        w4v = w4[:].rearrange("p (c k s) -> p c k s", c=4, k=4)
        t2 = work_pool.tile([128, 512], BF16, tag="t2")
        t2v = t2[:].rearrange("p (c k s) -> p c k s", c=4, k=2)
        nc.vector.tensor_tensor(t2v, w4v[:, :, 0:2, :], w4v[:, :, 2:4, :], ALU.add)

        # ---- out[(u,i), (c,:)] = sum_j w^T[j,i] hh[j,:]; col 128 = denom ----
        # the remaining k-pair sum rides on PSUM accumulation (2 matmuls).
        # pair-blocks live at 256-col offsets so no matmul output crosses
        # a 2 KiB PSUM bank.
        ops = opsum_pool.tile([128, 1024], F32, tag="ops")
        for l in range(8):
            c, u = l // 2, l % 2
            for h in range(2):
                nc.tensor.matmul(
                    ops[u * 64 : (u + 1) * 64, c * 256 : c * 256 + HHW],
                    lhsT=t2[u * 64 : (u + 1) * 64,
                            c * 128 + h * 64 : c * 128 + (h + 1) * 64],
                    rhs=hh[u * 64 : (u + 1) * 64, c * HHW : (c + 1) * HHW],
                    start=(h == 0), stop=(h == 1),
                    tile_position=(u * 64, u * 64),
                )

        # ---- numerator (bf16) + denominator (f32 bits smuggled into the
        #      same bf16 tile via bitcast) to HBM; host divides.
        #      evacuation split between ACT and DVE to balance load. ----
        osb = work_pool.tile([128, OW], BF16, tag="osb")
        nc.scalar.activation(
            osb[:, 0:256].rearrange("p (c w) -> p c w", c=2),
            ops[:, 0:512].rearrange("p (c z) -> p c z", c=2)[:, :, 0:128],
            ACTF.Copy)
        nc.vector.tensor_copy(
            osb[:, 256:512].rearrange("p (c w) -> p c w", c=2),
            ops[:, 512:1024].rearrange("p (c z) -> p c z", c=2)[:, :, 0:128])
        nc.scalar.activation(
            osb[:, 512:520].bitcast(F32),
            ops[:].rearrange("p (c z) -> p c z", c=4)[:, :, 128],
            ACTF.Copy)
        nc.sync.dma_start(out=out_d[q], in_=osb[:])


def build_nc():
    nc = bacc.Bacc("TRN2", target_bir_lowering=False, debug=False)
    in_d = nc.dram_tensor("inp", [OCTS, 128, INW], BF16, kind="ExternalInput").ap()
    aPat_d = nc.dram_tensor("apat", [128, 256], BF16, kind="ExternalInput").ap()
    out_d = nc.dram_tensor("out", [OCTS, 128, OW], BF16, kind="ExternalOutput").ap()
    with tile.TileContext(nc) as tc:
        _kernel_body(tc, in_d, aPat_d, out_d)
    nc.compile()
    return nc


def prep_inputs(hidden, adj, a):
    """Host-side packing: bf16 casts, transposed/interleaved layouts, shards."""
    bf = ml_dtypes.bfloat16
    hidden = np.asarray(hidden, dtype=np.float32)
    adj = np.asarray(adj)
    a = np.asarray(a, dtype=np.float32)

    hb = hidden.astype(bf)                                   # [B, 64, 128]

    # hT_q[q, d, l*64+i] = hidden[8q+l, i, d]
    hT = (hb.transpose(0, 2, 1)                              # [B, d, i]
          .reshape(B // 8, 8, D, N)                          # [q, l, d, i]
          .transpose(0, 2, 1, 3)                             # [q, d, l, i]
          .reshape(B // 8, D, 8 * N))

    # ind4_q[q, u*64+r, c*256+k*64+s] = (adj[8q+2c+u][s, r] == k+1)
    adjT = adj.transpose(0, 2, 1)                            # [b, r, s]
    onehot = (adjT[:, :, None, :] == np.arange(1, K + 1)[None, None, :, None]
              ).astype(bf)                                   # [b, r, k, s]
    ind4 = (onehot.reshape(B // 8, 4, 2, N, K, N)            # [q, c, u, r, k, s]
            .transpose(0, 2, 3, 1, 4, 5)                     # [q, u, r, c, k, s]
            .reshape(B // 8, 2 * N, 4 * K * N))

    # hh_oct[q][u*64+j, c*132+d] = hidden[8q+2c+u, j, d]; col 128 = 1
    hh = np.zeros((B, N, HHW), dtype=bf)
    hh[:, :, 0:D] = hb
    hh[:, :, D] = bf(1.0)
    hhq = (hh.reshape(B // 8, 4, 2, N, HHW)                  # [q, c, u, j, :]
           .transpose(0, 2, 3, 1, 4)                         # [q, u, j, c, :]
           .reshape(B // 8, 2 * N, 4 * HHW))

    packed = np.concatenate([hT, ind4, hhq], axis=2)         # [B//8, 128, INW]
    packed = np.ascontiguousarray(packed)

    # aPat[d, k*64+j] = a[k, d]
    aPat = np.ascontiguousarray(
        np.broadcast_to(a.T[:, :, None], (D, K, N)).reshape(D, K * N)
    ).astype(bf)

    in_maps = []
    for cidx in range(NCORES):
        qsl = slice(cidx * OCTS, (cidx + 1) * OCTS)
        in_maps.append({
            "inp": np.ascontiguousarray(packed[qsl]),
            "apat": aPat,
        })
    return in_maps


_NC_CACHE = {}


def run_device(hidden, adj, a, **spmd_kwargs):
    if "nc" not in _NC_CACHE:
        _NC_CACHE["nc"] = build_nc()
    nc = _NC_CACHE["nc"]
    in_maps = prep_inputs(hidden, adj, a)
    res = run_bass_kernel_spmd(nc, in_maps, list(range(NCORES)), **spmd_kwargs)
    raw = np.stack([res.results[c]["out"] for c in range(NCORES)], axis=0)
    raw = raw.reshape(NCORES * OCTS, 128, OW)                # [q, (u,i), :]
    num = raw[:, :, 0:512].astype(np.float32)                # [q, (u,i), (c,d)]
    den = np.ascontiguousarray(raw[:, :, 512:520]).view(np.float32)  # [q,(u,i),c]
    o = num.reshape(NCORES * OCTS, 2, N, 4, D).transpose(0, 3, 1, 2, 4)
    s = den.reshape(NCORES * OCTS, 2, N, 4).transpose(0, 3, 1, 2)
    out = (o / s[..., None]).reshape(B, N, D)
    return np.ascontiguousarray(out, dtype=np.float32), res


def kernel(hidden, adj, a):
    out, _ = run_device(hidden, adj, a)
    return out
